# revision 4
# baseline (speedup 1.0000x reference)
"""GCNII conv kernel v2 for 8 Trainium2 NeuronCores.

Key structure (vs baseline):
  - Dest nodes sharded by range: core c owns cols [c*6250, (c+1)*6250).
  - Dest tiles of width TW=96 within each core; edges per (tile, src-half)
    chunked into 128-edge chunks; schedule shared across cores (max pad).
  - Gathers issued per (group of tiles, half) on 4 SWDGE queues so up to 4
    Q7 pairs generate descriptors concurrently (dma_gather desc-gen is the
    machine bottleneck at ~6-8ns/idx serial per queue).
  - Selection matrices built in batched single-pass custom DVE op:
      S[e, b*TW + d] = (Idx == colp[e,b]) * nrm[e,b]
    (colp pre-offset by TW*batch-pos on host; f32 cols exact).
  - x0 added via identity-matmul into the segment PSUM; y = W_eff @ h via
    wide matmuls over groups of tiles.
"""

import os
import sys

sys.path.insert(0, "/opt/trn_rl_repo")

import numpy as np

N = 50000
D = 128
NCORES = 8
NPC = N // NCORES          # 6250 dest nodes per core
TW = 96                    # dest-tile width
TPC = (NPC + TW - 1) // TW   # 66 tiles per core
NPAD = TPC * TW             # 6336
HALF = N // 2
ALPHA = 0.1
THETA = 0.5
LAYER = 1
NQ = 4                     # SWDGE queues
Q0_WEIGHT = 0.75           # queue 0 shares the dispatcher Q7 pair
TPG = 3                    # tiles per group
KB = 32                    # S-build batch (chunks per custom-DVE op)
USE_CUSTOM_DVE = os.environ.get("KERNEL_NO_CUSTOM", "0") != "1"

_prog_cache = {}
LAST = None
_dve_op = None


def _get_custom_op():
    """Register (once) the fused one-hot*scale DVE op."""
    global _dve_op
    if _dve_op is not None:
        return _dve_op
    import concourse.dve_ops as dve_ops
    from concourse.dve_spec import Spec, Src0, Src1, eq, Idx, lower

    def _ref(in0, in1, c0, c1, c2):
        p = in0.shape[0]
        f0 = in0.reshape(p, -1).astype(np.float32)
        f1 = in1.reshape(p, -1).astype(np.float32)
        idx = np.arange(f0.shape[1], dtype=np.float32)[None, :]
        return (f0 == idx) * f1

    spec = Spec(body=eq(Idx, Src0) * Src1, reference=_ref)
    op = dve_ops.DveOp("ONEHOT_NRM_GCN", spec, subdim=False, uops_sha={})
    # register before sha pinning so opcode lookup works
    if op.name not in dve_ops._SUB_OPCODE_FOR_NAME:
        row = max(dve_ops._SUB_OPCODE_FOR_NAME.values()) + 1
        assert row < 0x20, row
        dve_ops.OPS.append(op)
        dve_ops._SUB_OPCODE_FOR_NAME[op.name] = row
        dve_ops.CUSTOM_DVE_SPECS[op.name] = spec
    # pin shas for both uop versions
    for ver in ("v3", "v4"):
        try:
            uops = lower(spec, ver=ver)
        except Exception:
            continue
        res = dve_ops.DveOpSpec(
            name=op.name,
            opcode=dve_ops.get_dve_sub_opcode(op.name),
            uops=uops,
            rd1_en=True,
        )
        op.uops_sha[ver] = res.sha(ver)
    _dve_op = op
    return op


def _wrap16(idx_list):
    w = idx_list.reshape(-1, 16).T.astype(np.int16)
    return np.tile(w, (8, 1))


def _build_program(schedule):
    """schedule: dict with per-tile chunk counts (shared across cores).

    schedule = {
      'Mlo': [TPC], 'Mhi': [TPC],  # chunks per (tile, half)
      'groups': [(t0, t1), ...],   # tile ranges per group
    }
    """
    import concourse.bacc as bacc
    import concourse.mybir as mybir
    import concourse.tile as tile
    from concourse import library_config

    f32 = mybir.dt.float32
    bf16 = mybir.dt.bfloat16
    i16 = mybir.dt.int16

    Mlo = schedule["Mlo"]
    Mhi = schedule["Mhi"]
    groups = schedule["groups"]
    TC = int(sum(Mlo) + sum(Mhi))
    CLO = int(sum(Mlo))
    CHI = int(sum(Mhi))

    op = _get_custom_op() if USE_CUSTOM_DVE else None

    nc = bacc.Bacc(
        "TRN2", target_bir_lowering=False, debug=False,
        num_devices=NCORES, num_swdge_queues=NQ,
    )
    xlo = nc.dram_tensor("xlo", [HALF, D], bf16, kind="ExternalInput").ap()
    xhi = nc.dram_tensor("xhi", [N - HALF, D], bf16, kind="ExternalInput").ap()
    ilo = nc.dram_tensor("ilo", [128, CLO * 8], i16, kind="ExternalInput").ap()
    ihi = nc.dram_tensor("ihi", [128, CHI * 8], i16, kind="ExternalInput").ap()
    colp = nc.dram_tensor("colp", [128, TC], f32, kind="ExternalInput").ap()
    nrm = nc.dram_tensor("nrm", [128, TC], f32, kind="ExternalInput").ap()
    iot = nc.dram_tensor("iot", [128, TW], bf16, kind="ExternalInput").ap()
    NPADv = len(Mlo) * TW
    x0t = nc.dram_tensor("x0t", [D, NPADv], bf16, kind="ExternalInput").ap()
    wl = nc.dram_tensor("wl", [D, D], bf16, kind="ExternalInput").ap()
    idm = nc.dram_tensor("idm", [D, D], bf16, kind="ExternalInput").ap()
    yt = nc.dram_tensor("yt", [D, NPADv], f32, kind="ExternalOutput").ap()

    with tile.TileContext(nc) as tc:
        with (
            tc.tile_pool(name="persist", bufs=1) as pp,
            tc.tile_pool(name="msl", bufs=6) as mpl,
            tc.tile_pool(name="msh", bufs=6) as mph,
            tc.tile_pool(name="sel", bufs=6) as sp,
            tc.tile_pool(name="hg", bufs=3) as hp,
            tc.tile_pool(name="x0g", bufs=3) as xp,
            tc.tile_pool(name="yg", bufs=2) as yp_pool,
            tc.tile_pool(name="pseg", bufs=6, space="PSUM") as psp,
            tc.tile_pool(name="py", bufs=2, space="PSUM") as pyp,
        ):
            nc.gpsimd.load_library(library_config.mlp)

            colp_sb = pp.tile([128, TC], f32)
            nrm_sb = pp.tile([128, TC], f32)
            wl_sb = pp.tile([128, 128], bf16)
            idm_sb = pp.tile([128, 128], bf16)
            iota_sb = pp.tile([128, TW], bf16)

            nc.sync.dma_start(colp_sb[:], colp[:, :])
            nc.sync.dma_start(nrm_sb[:], nrm[:, :])
            nc.sync.dma_start(wl_sb[:], wl[:, :])
            nc.sync.dma_start(idm_sb[:], idm[:, :])
            nc.sync.dma_start(iota_sb[:], iot[:, :])

            # offsets
            lo_base = np.concatenate([[0], np.cumsum(Mlo)]).astype(int)
            hi_base = np.concatenate([[0], np.cumsum(Mhi)]).astype(int)

            # size-greedy queue assignment (queue 0 weighted down, never
            # first): balances per-queue desc-gen chains
            calls = []
            for gi, (g0, g1) in enumerate(groups):
                calls.append((gi, 0, int(lo_base[g1] - lo_base[g0])))
                calls.append((gi, 1, int(hi_base[g1] - hi_base[g0])))
            qload = [0.0] * NQ
            qweight = [Q0_WEIGHT] + [1.0] * (NQ - 1)
            qassign = {}
            for (gi, hf, m) in calls:
                q = min(range(NQ), key=lambda i: qload[i] / qweight[i])
                qassign[(gi, hf)] = q
                qload[q] += m

            # per-group idx tiles (separate tiles -> fine-grained DMA deps)
            ilo_g_sb = []
            ihi_g_sb = []
            for (g0, g1) in groups:
                Mlo_g = int(lo_base[g1] - lo_base[g0])
                Mhi_g = int(hi_base[g1] - hi_base[g0])
                tl = pp.tile([128, max(Mlo_g, 1) * 8], i16, name=f"ilo_g{g0}")
                th = pp.tile([128, max(Mhi_g, 1) * 8], i16, name=f"ihi_g{g0}")
                if Mlo_g:
                    nc.sync.dma_start(
                        tl[:], ilo[:, lo_base[g0] * 8 : lo_base[g1] * 8]
                    )
                if Mhi_g:
                    nc.sync.dma_start(
                        th[:], ihi[:, hi_base[g0] * 8 : hi_base[g1] * 8]
                    )
                ilo_g_sb.append(tl)
                ihi_g_sb.append(th)

            # consumption order S-column index per (tile): lo chunks then hi
            ci_of_tile = {}
            ci = 0
            for t in range(len(Mlo)):
                ci_of_tile[t] = ci
                ci += int(Mlo[t]) + int(Mhi[t])
            assert ci == TC

            alt = [0]  # alternate copies between DVE and Act

            for gi, (g0, g1) in enumerate(groups):
                ng = g1 - g0
                Mlo_g = int(lo_base[g1] - lo_base[g0])
                Mhi_g = int(hi_base[g1] - hi_base[g0])

                x0g = xp.tile([128, ng * TW], bf16, tag="x0")
                nc.sync.dma_start(x0g[:], x0t[:, g0 * TW : g1 * TW])

                msl = mpl.tile([128, max(Mlo_g, 1), 128], bf16, tag="msl")
                if Mlo_g:
                    nc.gpsimd.dma_gather(
                        msl[:, :, :],
                        xlo[:, :],
                        ilo_g_sb[gi][:, :],
                        Mlo_g * 128,
                        Mlo_g * 128,
                        D,
                        single_packet=False,
                        queue_num=qassign[(gi, 0)],
                    )
                msh = mph.tile([128, max(Mhi_g, 1), 128], bf16, tag="msh")
                if Mhi_g:
                    nc.gpsimd.dma_gather(
                        msh[:, :, :],
                        xhi[:, :],
                        ihi_g_sb[gi][:, :],
                        Mhi_g * 128,
                        Mhi_g * 128,
                        D,
                        single_packet=False,
                        queue_num=qassign[(gi, 1)],
                    )

                # S-batches over the group's consumption-order column range
                ci0 = ci_of_tile[g0]
                ci1 = ci_of_tile[g1 - 1] + int(Mlo[g1 - 1]) + int(Mhi[g1 - 1])
                n_cols = ci1 - ci0
                s_tiles = {}
                for b0 in range(0, n_cols, KB):
                    bk = min(KB, n_cols - b0)
                    Sb = sp.tile([128, bk * TW], bf16, tag="sel")
                    if op is not None:
                        nc.vector._custom_dve(
                            op,
                            out=Sb[:].rearrange("p (b w) -> p b w", w=TW),
                            in0=colp_sb[:, ci0 + b0 : ci0 + b0 + bk, None]
                            .to_broadcast((128, bk, TW)),
                            in1=nrm_sb[:, ci0 + b0 : ci0 + b0 + bk, None]
                            .to_broadcast((128, bk, TW)),
                        )
                    else:
                        nc.vector.tensor_tensor(
                            out=Sb[:].rearrange("p (b w) -> p b w", w=TW),
                            in0=iot[None] if False else iota_sb[:, None, :]
                            .to_broadcast((128, bk, TW)),
                            in1=colp_sb[:, ci0 + b0 : ci0 + b0 + bk, None]
                            .to_broadcast((128, bk, TW)),
                            op=mybir.AluOpType.is_equal,
                        )
                        nc.vector.tensor_tensor(
                            out=Sb[:].rearrange("p (b w) -> p b w", w=TW),
                            in0=Sb[:].rearrange("p (b w) -> p b w", w=TW),
                            in1=nrm_sb[:, ci0 + b0 : ci0 + b0 + bk, None]
                            .to_broadcast((128, bk, TW)),
                            op=mybir.AluOpType.mult,
                        )
                    for k in range(bk):
                        s_tiles[ci0 + b0 + k] = (Sb, k)

                hgt = hp.tile([128, ng * TW], bf16, tag="hg")
                for t in range(g0, g1):
                    ps = psp.tile([128, TW], f32, space="PSUM", tag="ps")
                    ci_t = ci_of_tile[t]
                    nch = int(Mlo[t]) + int(Mhi[t])
                    for j in range(nch):
                        if j < int(Mlo[t]):
                            src = msl[:, lo_base[t] - lo_base[g0] + j, :]
                        else:
                            jj = j - int(Mlo[t])
                            src = msh[:, hi_base[t] - hi_base[g0] + jj, :]
                        Sb, k = s_tiles[ci_t + j]
                        nc.tensor.matmul(
                            ps[:],
                            lhsT=src,
                            rhs=Sb[:, k * TW : (k + 1) * TW],
                            start=(j == 0),
                            stop=False,
                        )
                    nc.tensor.matmul(
                        ps[:],
                        lhsT=idm_sb[:],
                        rhs=x0g[:, (t - g0) * TW : (t - g0 + 1) * TW],
                        start=(nch == 0),
                        stop=True,
                    )
                    # copy psum -> hg slice (alternate engine)
                    dst = hgt[:, (t - g0) * TW : (t - g0 + 1) * TW]
                    if alt[0] % 2 == 0:
                        nc.vector.tensor_copy(dst, ps[:])
                    else:
                        nc.scalar.copy(dst, ps[:])
                    alt[0] += 1

                # TW matmul over the whole group (ng*TW <= 512)
                ygt = yp_pool.tile([128, ng * TW], f32, tag="yg")
                pyt = pyp.tile([128, ng * TW], f32, space="PSUM", tag="py")
                nc.tensor.matmul(
                    pyt[:], lhsT=wl_sb[:], rhs=hgt[:], start=True, stop=True
                )
                if alt[0] % 2 == 0:
                    nc.vector.tensor_copy(ygt[:], pyt[:])
                else:
                    nc.scalar.copy(ygt[:], pyt[:])
                alt[0] += 1
                nc.sync.dma_start(yt[:, g0 * TW : g1 * TW], ygt[:])

    nc.compile()
    return nc


def _preprocess(x, x0, edge_index, norm, Wm):
    row = np.ascontiguousarray(edge_index[0]).astype(np.int64)
    col = np.ascontiguousarray(edge_index[1]).astype(np.int64)
    norm = np.ascontiguousarray(norm).astype(np.float32)
    x = np.ascontiguousarray(x).astype(np.float32)
    x0 = np.ascontiguousarray(x0).astype(np.float32)
    Wm = np.ascontiguousarray(Wm).astype(np.float32)

    import ml_dtypes

    bf = ml_dtypes.bfloat16

    core = col // NPC
    is_hi = row >= HALF
    nscaled = (1.0 - ALPHA) * norm

    # variable-width tiles: cut when lo/hi degree-sum would exceed 4 chunks
    CAP = 512
    dlo_deg = np.bincount(col[row < HALF], minlength=N)
    dhi_deg = np.bincount(col[row >= HALF], minlength=N)
    bounds_all = []
    tpcs = []
    for c in range(NCORES):
        d0, d1 = c * NPC, (c + 1) * NPC
        bounds = [d0]
        lo = hi = nd = 0
        for d in range(d0, d1):
            if lo + dlo_deg[d] > CAP or hi + dhi_deg[d] > CAP or nd >= TW:
                bounds.append(d)
                lo = hi = nd = 0
            lo += int(dlo_deg[d]); hi += int(dhi_deg[d]); nd += 1
        bounds.append(d1)
        bounds_all.append(bounds)
        tpcs.append(len(bounds) - 1)
    TPCv = max(tpcs)
    for c in range(NCORES):
        while len(bounds_all[c]) < TPCv + 1:
            bounds_all[c].append((c + 1) * NPC)
    tloc = np.zeros_like(col)
    for c in range(NCORES):
        m = core == c
        tloc[m] = np.searchsorted(bounds_all[c], col[m], side="right") - 1

    # sort by (core, tile, half, col) -- stable ordering for chunking
    order = np.lexsort((col, is_hi, tloc, core))
    rs = row[order]
    cs = col[order]
    ns = nscaled[order]
    core_s = core[order]
    t_s = tloc[order]
    h_s = is_hi[order]

    # per (core, tile, half) edge count
    key = (core_s * TPCv + t_s) * 2 + h_s
    cnt = np.bincount(key, minlength=NCORES * TPCv * 2).reshape(NCORES, TPCv, 2)
    Mct = -(-cnt // 128)  # ceil chunks per (core, tile, half)
    Mlo = Mct[:, :, 0].max(axis=0)
    Mhi = Mct[:, :, 1].max(axis=0)

    # groups of TPG tiles
    # taper: tiny first group (fast ramp: first gather's desc-gen blocks
    # dispatch) and tiny last group (short tail)
    gb = [0, 1]
    while gb[-1] < TPCv - 2:
        gb.append(min(gb[-1] + TPG, TPCv - 2))
    gb += [TPCv - 1, TPCv]
    groups = list(zip(gb[:-1], gb[1:]))

    schedule = {
        "Mlo": tuple(int(v) for v in Mlo),
        "Mhi": tuple(int(v) for v in Mhi),
        "groups": tuple(groups),
    }

    CLO = int(Mlo.sum())
    CHI = int(Mhi.sum())
    TC = CLO + CHI

    beta = np.float32(np.log(THETA / LAYER + 1.0))
    W_eff = (1.0 - beta) * np.eye(D, dtype=np.float32) + beta * Wm
    wl = np.ascontiguousarray(W_eff.T).astype(bf)
    idm = np.eye(D, dtype=np.float32).astype(bf)
    iot = np.ascontiguousarray(
        np.tile(np.arange(TW, dtype=np.float32)[None, :], (128, 1))
    ).astype(bf)
    xlo = np.ascontiguousarray(x[:HALF]).astype(bf)
    xhi = np.ascontiguousarray(x[HALF:]).astype(bf)

    # boundaries of each (core,tile,half) run in the sorted arrays
    starts = np.zeros(NCORES * TPCv * 2 + 1, dtype=np.int64)
    np.cumsum(cnt.reshape(-1), out=starts[1:])

    lo_base = np.concatenate([[0], np.cumsum(Mlo)]).astype(int)
    hi_base = np.concatenate([[0], np.cumsum(Mhi)]).astype(int)

    # consumption-order column base per tile
    ci_of_tile = np.zeros(TPCv + 1, dtype=np.int64)
    np.cumsum(Mlo + Mhi, out=ci_of_tile[1:])

    in_maps = []
    for c in range(NCORES):
        ilo_a = np.zeros((128, CLO * 8), dtype=np.int16)
        ihi_a = np.zeros((128, CHI * 8), dtype=np.int16)
        colp_a = np.full((128, TC), -1.0, dtype=np.float32)
        nrm_a = np.zeros((128, TC), dtype=np.float32)
        x0t = np.zeros((D, TPCv * TW), dtype=np.float32)
        d0 = c * NPC
        bnd = bounds_all[c]
        for t in range(TPCv):
            b0, b1 = bnd[t], bnd[t + 1]
            if b1 > b0:
                x0t[:, t * TW : t * TW + (b1 - b0)] = (ALPHA * x0[b0:b1]).T

        for t in range(TPCv):
            for hf, (M, ia, base) in enumerate(
                ((int(Mlo[t]), ilo_a, lo_base[t]), (int(Mhi[t]), ihi_a, hi_base[t]))
            ):
                if M == 0:
                    continue
                k = (c * TPCv + t) * 2 + hf
                e0, e1 = int(starts[k]), int(starts[k + 1])
                n_e = e1 - e0
                pi = np.zeros(M * 128, dtype=np.int64)
                pc = np.full(M * 128, -1.0, dtype=np.float32)
                pn = np.zeros(M * 128, dtype=np.float32)
                pi[:n_e] = rs[e0:e1] - (HALF if hf else 0)
                pc[:n_e] = (cs[e0:e1] - bnd[t]).astype(np.float32)
                pn[:n_e] = ns[e0:e1]
                ia[:, base * 8 : (base + M) * 8] = _wrap16(pi)
                # consumption-order columns: tile t chunk j -> ci_of_tile[t]+ (hf? Mlo[t]:0) + j
                cbase = int(ci_of_tile[t]) + (int(Mlo[t]) if hf else 0)
                colp_a[:, cbase : cbase + M] = pc.reshape(M, 128).T
                nrm_a[:, cbase : cbase + M] = pn.reshape(M, 128).T

        # add TW*batch-position offsets for the fused Idx comparison
        if USE_CUSTOM_DVE:
            # batches are per group, KB chunks each, offsets = TW * (pos in batch)
            for (g0, g1) in groups:
                ci0 = int(ci_of_tile[g0])
                ci1 = int(ci_of_tile[g1])
                n_cols = ci1 - ci0
                for b0 in range(0, n_cols, KB):
                    bk = min(KB, n_cols - b0)
                    offs = (np.arange(bk) * TW).astype(np.float32)
                    blk = colp_a[:, ci0 + b0 : ci0 + b0 + bk]
                    # padded (-1) stays out of range: -1 + TW*k never equals
                    # Idx in [TW*k, TW*(k+1)) except -1+TW*(k+1) = TW*k+TW-1-TW...
                    # careful: -1 + offs_next could alias; keep pads at -1e9
                    blk[blk < 0] = -1e9
                    blk += offs[None, :]
                    colp_a[:, ci0 + b0 : ci0 + b0 + bk] = blk

        in_maps.append(
            {
                "xlo": xlo,
                "xhi": xhi,
                "ilo": ilo_a,
                "ihi": ihi_a,
                "colp": colp_a,
                "nrm": nrm_a,
                "iot": iot,
                "x0t": np.ascontiguousarray(x0t).astype(bf),
                "wl": wl,
                "idm": idm,
                "yt": None,
            }
        )
        in_maps[-1].pop("yt")
    return schedule, in_maps, bounds_all


def kernel(x, x0, edge_index, norm, W):
    global LAST
    from concourse.bass_utils import run_bass_kernel_spmd

    schedule, in_maps, bounds_all = _preprocess(x, x0, edge_index, norm, W)
    key = (schedule["Mlo"], schedule["Mhi"], schedule["groups"])
    if key not in _prog_cache:
        _prog_cache[key] = _build_program(schedule)
    nc = _prog_cache[key]

    trace = os.environ.get("KERNEL_TRACE", "0") == "1"
    res = run_bass_kernel_spmd(
        nc,
        in_maps,
        core_ids=list(range(NCORES)),
        trace=trace,
    )
    LAST = res

    y = np.empty((N, D), dtype=np.float32)
    for c in range(NCORES):
        ytc = res.results[c]["yt"]
        bnd = bounds_all[c]
        for t in range(len(bnd) - 1):
            b0, b1 = bnd[t], bnd[t + 1]
            if b1 > b0:
                y[b0:b1] = ytc[:, t * TW : t * TW + (b1 - b0)].T
    return y


# keep the reference-compatible signature name `TW` for kwargs call
def kernel_entry(**inputs):
    return kernel(
        inputs["x"], inputs["x0"], inputs["edge_index"], inputs["norm"], inputs["W"]
    )
